# revision 1
# baseline (speedup 1.0000x reference)
"""Trainium2 Bass kernel for nn_DDH_49246095016535 (dense CNN + LC + FC + fuse).

Strategy: pure data parallelism over 8 NeuronCores (32 samples each).
Training-mode BN statistics are made exact via a per-layer AllGather of
per-channel partial (sum, sumsq) followed by a local combine on every core.
Convs run as PE-tile-packed matmuls ((kw, cin) on the contraction dim, kh
accumulated in PSUM); maxpool commutes with the per-channel affine BN + ReLU
(scale >= 0 here), so pooling runs on raw conv outputs and BN+ReLU is applied
once on the pooled tensor. The FC layer is decomposed per spatial position so
conv outputs feed the TensorEngine without any transposes.
"""
import sys

sys.path.insert(0, '/opt/trn_rl_repo')

import numpy as np
import ml_dtypes

import concourse.bass as bass
import concourse.tile as tile
import concourse.mybir as mybir

F32 = mybir.dt.float32
BF16 = mybir.dt.bfloat16
NPBF16 = ml_dtypes.bfloat16

N_CORES = 8
BL = 32          # samples per core
EPS = 1e-5

N1 = 256 * 62 * 62
N2 = 256 * 30 * 30
N3 = 256 * 14 * 14
N4 = 256 * 36
N5 = 256

AF = mybir.ActivationFunctionType
ALU = mybir.AluOpType
AX = mybir.AxisListType

MAX_DRAIN_WAITS = 1


def _patched_drain_and_barrier(self, tick_clock, wait_clock):
    from concourse.vector_clock import ScopedClock
    nc = self.nc
    drain_inst = nc.sync.drain()
    wait_clock.add_sem_waits(drain_inst.ins, ScopedClock({None: tick_clock.global_clock}))
    si = drain_inst.ins.sync_info
    if si is not None and len(si.on_wait) > MAX_DRAIN_WAITS:
        waits = list(si.on_wait)
        drain_inst.ins.sync_info = mybir.SyncInfo(
            on_wait=waits[:MAX_DRAIN_WAITS], on_update=list(si.on_update))
        for k in range(MAX_DRAIN_WAITS, len(waits), MAX_DRAIN_WAITS):
            extra = nc.sync.drain()
            extra.ins.sync_info = mybir.SyncInfo(
                on_wait=waits[k:k + MAX_DRAIN_WAITS], on_update=[])
    nc.all_engine_barrier()
    assert self.sems is not None
    popped = nc._tile_sem_poison_stack.pop()
    assert popped is self._sem_poison
    nc.clear_and_free_semaphores(list(self.sems.allocated().values()))
    nc.all_engine_barrier()


tile.TileContext._drain_and_barrier = _patched_drain_and_barrier


def _split_excess_waits(nc, limit=1):
    """The neuronxcc walrus codegen accepts at most one sync-wait per
    instruction; Tile's wait assigner can attach several. Move the excess
    onto same-engine NoOps inserted immediately before the instruction."""
    nid = 0
    for f in nc.m.functions:
        for b in f.blocks:
            insts = b.instructions
            new_list = []
            changed = False
            for inst in insts:
                si = getattr(inst, "sync_info", None)
                if si is not None and len(si.on_wait) > limit and inst.engine is not None:
                    waits = list(si.on_wait)
                    keep, excess = waits[:limit], waits[limit:]
                    inst.sync_info = mybir.SyncInfo(
                        on_wait=keep, on_update=list(si.on_update))
                    for k in range(0, len(excess), limit):
                        nop = mybir.InstNoOp(name=f"I-wsplit-{nid}", ins=[], outs=[])
                        nid += 1
                        nop.engine = inst.engine
                        nop.sync_info = mybir.SyncInfo(
                            on_wait=excess[k:k + limit], on_update=[])
                        new_list.append(nop)
                    changed = True
                new_list.append(inst)
            if changed:
                insts[:] = new_list
    return nc


def _stats_allgather(nc, pool, name, part_n, s1, s2, fold_groups):
    """Per-partition partial (S1,S2) -> AllGather over 8 cores -> global sums
    broadcast back into every fold-group's partition rows.
    Returns gstat [128, 2] f32."""
    st = pool.tile([128, 2], F32, name=f"st_{name}")
    nc.vector.tensor_copy(out=st[:, 0:1], in_=s1)
    nc.vector.tensor_copy(out=st[:, 1:2], in_=s2)
    G = len(fold_groups)
    cc_in = nc.dram_tensor(f"cc_{name}_in", [G, part_n, 2], F32)
    cc_out = nc.dram_tensor(f"cc_{name}_out", [N_CORES, G, part_n, 2], F32,
                            addr_space="Shared")
    for k, base in enumerate(fold_groups):
        nc.sync.dma_start(cc_in[k], st[base:base + part_n])
    nc.gpsimd.collective_compute(
        "AllGather", ALU.bypass,
        replica_groups=[list(range(N_CORES))],
        ins=[cc_in[:]], outs=[cc_out[:]],
    )
    # gall[c, s, r, g] <- cc_out[r, g, c, s], replicated into every group base
    gall = pool.tile([128, 2, N_CORES, G], F32, name=f"gall_{name}")
    src = bass.AP(tensor=cc_out, offset=0,
                  ap=[[2, part_n], [1, 2], [G * part_n * 2, N_CORES],
                      [part_n * 2, G]])
    for base in fold_groups:
        nc.sync.dma_start(gall[base:base + part_n], src)
    gstat = pool.tile([128, 2], F32, name=f"gstat_{name}")
    nc.vector.reduce_sum(gstat[:], gall[:], axis=AX.XY)
    return gstat


def _bn_scale_shift(nc, pool, name, gstat, bnp, n, eps_t):
    """gstat [128,2] raw (S1,S2); bnp [128,2] (gamma, beta).
    Returns (scale [128,1], shift [128,1]) f32."""
    mean = pool.tile([128, 1], F32, name=f"mean_{name}")
    var = pool.tile([128, 1], F32, name=f"var_{name}")
    tmp = pool.tile([128, 1], F32, name=f"tmp_{name}")
    scale = pool.tile([128, 1], F32, name=f"scale_{name}")
    shift = pool.tile([128, 1], F32, name=f"shift_{name}")
    inv_n = 1.0 / n
    nc.vector.tensor_scalar_mul(mean[:], gstat[:, 0:1], inv_n)
    nc.vector.tensor_scalar_mul(var[:], gstat[:, 1:2], inv_n)
    nc.vector.tensor_mul(tmp[:], mean[:], mean[:])
    nc.vector.tensor_sub(var[:], var[:], tmp[:])
    nc.scalar.activation(out=tmp[:], in_=var[:], func=AF.Sqrt,
                         bias=eps_t[:], scale=1.0)
    nc.vector.reciprocal(out=tmp[:], in_=tmp[:])
    nc.vector.tensor_mul(scale[:], bnp[:, 0:1], tmp[:])
    nc.vector.tensor_mul(tmp[:], mean[:], scale[:])
    nc.vector.tensor_sub(shift[:], bnp[:, 1:2], tmp[:])
    return scale, shift



def _open_pool(tc, **kw):
    cm = tc.tile_pool(**kw)
    return cm, cm.__enter__()


def build_nc():
    nc = bass.Bass("TRN2", num_devices=N_CORES)

    im1_d = nc.dram_tensor("im1", [4, 9, 8, 64, 62], BF16, kind="ExternalInput")
    w1_d = nc.dram_tensor("w1t", [9, 3, 20], BF16, kind="ExternalInput")
    w2_d = nc.dram_tensor("w2t", [40, 2, 40], BF16, kind="ExternalInput")
    w3_d = nc.dram_tensor("w3t", [80, 2, 60], BF16, kind="ExternalInput")
    lcw_d = nc.dram_tensor("lcwt", [120, 36, 2, 80], BF16, kind="ExternalInput")
    lcb_d = nc.dram_tensor("lcb", [80, 36], F32, kind="ExternalInput")
    fw1a_d = nc.dram_tensor("fcw1a", [60, 25, 768], BF16, kind="ExternalInput")
    fw1b_d = nc.dram_tensor("fcw1b", [60, 24, 768], BF16, kind="ExternalInput")
    fw2a_d = nc.dram_tensor("fcw2a", [40, 36, 768], BF16, kind="ExternalInput")
    fw2b_d = nc.dram_tensor("fcw2b", [40, 36, 768], BF16, kind="ExternalInput")
    b1_d = nc.dram_tensor("b1v", [128, 1], F32, kind="ExternalInput")
    b2_d = nc.dram_tensor("b2v", [128, 1], F32, kind="ExternalInput")
    b3_d = nc.dram_tensor("b3v", [128, 1], F32, kind="ExternalInput")
    bnp1_d = nc.dram_tensor("bnp1", [128, 2], F32, kind="ExternalInput")
    bnp2_d = nc.dram_tensor("bnp2", [128, 2], F32, kind="ExternalInput")
    bnp3_d = nc.dram_tensor("bnp3", [128, 2], F32, kind="ExternalInput")
    bnp4_d = nc.dram_tensor("bnp4", [128, 2], F32, kind="ExternalInput")
    fcb4_d = nc.dram_tensor("fcb4", [4, 192], F32, kind="ExternalInput")
    bn5p_d = nc.dram_tensor("bn5p", [4, 2, 192], F32, kind="ExternalInput")
    fw4_d = nc.dram_tensor("fw4", [128, 192], F32, kind="ExternalInput")
    fb4_d = nc.dram_tensor("fb4", [128, 12], F32, kind="ExternalInput")
    ones4_d = nc.dram_tensor("ones4", [128, 4], F32, kind="ExternalInput")
    out_d = nc.dram_tensor("out", [BL, 48], F32, kind="ExternalOutput")

    with tile.TileContext(nc) as tc:
        persist_cm, persist = _open_pool(tc, name="persist", bufs=1)
        chunks_cm, chunks = _open_pool(tc, name="chunks", bufs=6)
        psum_conv_cm, psum_conv = _open_pool(tc, name="psconv", bufs=2, space="PSUM")

        # ---------------- persistent params ----------------
        eps_t = persist.tile([128, 1], F32)
        nc.vector.memset(eps_t[:], EPS)
        b1v = persist.tile([128, 1], F32)
        nc.sync.dma_start(b1v[:], b1_d[:])
        b2v = persist.tile([128, 1], F32)
        nc.sync.dma_start(b2v[:], b2_d[:])
        b3v = persist.tile([128, 1], F32)
        nc.sync.dma_start(b3v[:], b3_d[:])
        bnp1 = persist.tile([128, 2], F32)
        nc.sync.dma_start(bnp1[:], bnp1_d[:])
        bnp2 = persist.tile([128, 2], F32)
        nc.sync.dma_start(bnp2[:], bnp2_d[:])
        bnp3 = persist.tile([128, 2], F32)
        nc.sync.dma_start(bnp3[:], bnp3_d[:])
        bnp4 = persist.tile([128, 2], F32)
        nc.sync.dma_start(bnp4[:], bnp4_d[:])
        lcb = persist.tile([80, 36], F32)
        nc.sync.dma_start(lcb[:], lcb_d[:])
        fcb4 = persist.tile([4, 192], F32)
        nc.sync.dma_start(fcb4[:], fcb4_d[:])
        bn5p = persist.tile([4, 2, 192], F32)
        nc.sync.dma_start(bn5p[:], bn5p_d[:])
        fw4 = persist.tile([128, 192], F32)
        nc.sync.dma_start(fw4[:], fw4_d[:])
        fb4 = persist.tile([128, 12], F32)
        nc.sync.dma_start(fb4[:], fb4_d[:])
        ones4 = persist.tile([128, 4], F32)
        nc.sync.dma_start(ones4[:], ones4_d[:])

        w1t = persist.tile([128, 3, 20], BF16)
        for g in range(4):
            nc.sync.dma_start(w1t[32 * g:32 * g + 9], w1_d[:])
        w2t = persist.tile([128, 2, 40], BF16)
        nc.sync.dma_start(w2t[0:40], w2_d[:])
        nc.sync.dma_start(w2t[64:104], w2_d[:])
        w3t = persist.tile([128, 2, 60], BF16)
        nc.sync.dma_start(w3t[0:80], w3_d[:])

        # persistent activations / stats
        pooled2 = persist.tile([128, 2, 8, 15, 15], BF16)   # p=64ct+m, rg, bsub
        pooled3 = persist.tile([128, 8, 2, 7, 7], BF16)     # p=64g2+o, w, b01
        h3c = persist.tile([128, 32, 49], BF16)             # c rows 0-59 & 64-123
        h3r = persist.tile([128, 32, 7, 6], BF16)            # rows dw*60+c
        lc_raw = persist.tile([128, 36, BL], BF16)           # rows o<80
        lc_bn = persist.tile([128, 36, BL], BF16)
        lc_sq = persist.tile([128, 36, BL], BF16)
        s1a_1 = persist.tile([128, 64], F32)
        s2a_1 = persist.tile([128, 64], F32)
        s1a_2 = persist.tile([128, 32], F32)
        s2a_2 = persist.tile([128, 32], F32)
        s1a_3 = persist.tile([128, 8], F32)
        s2a_3 = persist.tile([128, 8], F32)
        s1f = persist.tile([128, 1], F32)
        s2f = persist.tile([128, 1], F32)

        lcw_cm, lcw_pool = _open_pool(tc, name="lcwpool", bufs=1)
        lcw = lcw_pool.tile([128, 36, 2, 80], BF16)
        nc.sync.dma_start(lcw[0:120], lcw_d[:])

        # ================= conv1 =================
        pool1_cm, pool1_pool = _open_pool(tc, name="pool1pool", bufs=1, side="right")
        pooled1 = pool1_pool.tile([128, 4, 2, 31, 31], BF16)  # p=32j+c, g, b01

        im1_cm, im1_pool = _open_pool(tc, name="im1pool", bufs=1, side="right")
        im1 = im1_pool.tile([128, 8, 64, 62], BF16)
        for g in range(4):
            nc.sync.dma_start(im1[32 * g:32 * g + 9], im1_d[g])

        for b01 in range(2):
            for blk in range(8):
                w_idx = b01 * 8 + blk
                rows = 8 if blk < 7 else 6
                n_free = rows * 62
                banks = [psum_conv.tile([128, 496], F32, tag=f"pb{i}",
                                        name=f"c1b{i}_{w_idx}") for i in range(4)]
                for i in range(4):
                    for j in range(4):
                        b = 2 * j + b01
                        for s in range(3):
                            nc.tensor.matmul(
                                banks[i][32 * j:32 * j + 20, :n_free],
                                lhsT=w1t[32 * i:32 * i + 9, s, :],
                                rhs=im1[32 * i:32 * i + 9, b,
                                        blk * 8 + s:blk * 8 + s + rows, :],
                                start=(s == 0), stop=(s == 2),
                                tile_position=(32 * i, 32 * j),
                            )
                for i in range(4):
                    ch = w_idx * 4 + i
                    ych = chunks.tile([128, 8, 62], BF16, tag="ych",
                                      name=f"y1ch_{ch}")
                    nc.scalar.activation(
                        out=ych[:, :rows, :],
                        in_=banks[i][:, :n_free].rearrange(
                            "p (a b) -> p a b", a=rows),
                        func=AF.Identity, bias=b1v[:], scale=1.0,
                        accum_out=s1a_1[:, ch:ch + 1])
                    sq = chunks.tile([128, 8, 62], BF16, tag="ysq",
                                     name=f"y1sq_{ch}")
                    nc.vector.tensor_mul(sq[:, :rows, :], ych[:, :rows, :],
                                         ych[:, :rows, :])
                    nc.vector.reduce_sum(s2a_1[:, ch:ch + 1], sq[:, :rows, :],
                                         axis=AX.XY)
                    p1 = chunks.tile([128, 8, 31], BF16, tag="yp1",
                                     name=f"y1p1_{ch}")
                    nc.vector.tensor_max(
                        out=p1[:, :rows, :],
                        in0=ych[:, :rows, 0:62:2], in1=ych[:, :rows, 1:62:2])
                    nc.vector.tensor_max(
                        out=pooled1[:, i, b01, blk * 4:blk * 4 + rows // 2, :],
                        in0=p1[:, 0:rows:2, :], in1=p1[:, 1:rows:2, :])

        nc.vector.reduce_sum(s1f[:], s1a_1[:], axis=AX.X)
        nc.vector.reduce_sum(s2f[:], s2a_1[:], axis=AX.X)
        gstat1 = _stats_allgather(nc, persist, "bn1", 20, s1f[:], s2f[:],
                                  fold_groups=[0, 32, 64, 96])
        sc1, sh1 = _bn_scale_shift(nc, persist, "bn1", gstat1, bnp1, N1, eps_t)
        nc.scalar.activation(out=pooled1[:], in_=pooled1[:], func=AF.Relu,
                             bias=sh1[:], scale=sc1[:])

        # free im1 space; begin FC x1 weight load into it
        im1_cm.__exit__(None, None, None)
        fcw1_cm, fcw1_pool = _open_pool(tc, name="fcw1pool", bufs=1)
        fcw1 = fcw1_pool.tile([128, 25, 768], BF16)
        nc.sync.dma_start(fcw1[0:60, 0:25], fw1a_d[:])
        nc.sync.dma_start(fcw1[64:124, 0:24], fw1b_d[:])

        # ================= conv2 =================
        im2_cm, im2_pool = _open_pool(tc, name="im2pool", bufs=1)
        im2 = im2_pool.tile([128, 16, 31, 30], BF16)
        for rg in range(2):
            for dw in range(2):
                for g in (2 * rg, 2 * rg + 1):
                    for j in range(4):
                        b0 = 8 * g + 2 * j - 16 * rg
                        nc.sync.dma_start(
                            im2[64 * rg + 20 * dw:64 * rg + 20 * dw + 20,
                                b0:b0 + 2, :, :],
                            pooled1[32 * j:32 * j + 20, g, :, :, dw:dw + 30])
        pool1_cm.__exit__(None, None, None)

        for bsub in range(8):
            for h in range(2):
                w_idx = bsub * 2 + h
                rows = 16 if h == 0 else 14
                n_free = rows * 30
                banks = [psum_conv.tile([128, 480], F32, tag=f"pb{i}",
                                        name=f"c2b{i}_{w_idx}") for i in range(2)]
                for rg in range(2):
                    for ct in range(2):
                        b = 8 * ct + bsub
                        for s in range(2):
                            nc.tensor.matmul(
                                banks[rg][64 * ct:64 * ct + 40, :n_free],
                                lhsT=w2t[64 * rg:64 * rg + 40, s, :],
                                rhs=im2[64 * rg:64 * rg + 40, b,
                                        h * 16 + s:h * 16 + s + rows, :],
                                start=(s == 0), stop=(s == 1),
                                tile_position=(64 * rg, 64 * ct),
                            )
                for rg in range(2):
                    ch = w_idx * 2 + rg
                    ych = chunks.tile([128, 16, 30], BF16, tag="ych",
                                      name=f"y2ch_{ch}")
                    nc.scalar.activation(
                        out=ych[:, :rows, :],
                        in_=banks[rg][:, :n_free].rearrange(
                            "p (a b) -> p a b", a=rows),
                        func=AF.Identity, bias=b2v[:], scale=1.0,
                        accum_out=s1a_2[:, ch:ch + 1])
                    sq = chunks.tile([128, 16, 30], BF16, tag="ysq",
                                     name=f"y2sq_{ch}")
                    nc.vector.tensor_mul(sq[:, :rows, :], ych[:, :rows, :],
                                         ych[:, :rows, :])
                    nc.vector.reduce_sum(s2a_2[:, ch:ch + 1], sq[:, :rows, :],
                                         axis=AX.XY)
                    p1 = chunks.tile([128, 16, 15], BF16, tag="yp1",
                                     name=f"y2p1_{ch}")
                    nc.vector.tensor_max(
                        out=p1[:, :rows, :],
                        in0=ych[:, :rows, 0:30:2], in1=ych[:, :rows, 1:30:2])
                    nc.vector.tensor_max(
                        out=pooled2[:, rg, bsub, h * 8:h * 8 + rows // 2, :],
                        in0=p1[:, 0:rows:2, :], in1=p1[:, 1:rows:2, :])

        im2_cm.__exit__(None, None, None)
        fcw2_cm, fcw2_pool = _open_pool(tc, name="fcw2pool", bufs=1, side="right")
        fcw2 = fcw2_pool.tile([128, 36, 768], BF16)
        nc.sync.dma_start(fcw2[0:40], fw2a_d[:])
        nc.sync.dma_start(fcw2[64:104], fw2b_d[:])
        nc.vector.reduce_sum(s1f[:], s1a_2[:], axis=AX.X)
        nc.vector.reduce_sum(s2f[:], s2a_2[:], axis=AX.X)
        gstat2 = _stats_allgather(nc, persist, "bn2", 40, s1f[:], s2f[:],
                                  fold_groups=[0, 64])
        sc2, sh2 = _bn_scale_shift(nc, persist, "bn2", gstat2, bnp2, N2, eps_t)
        nc.scalar.activation(out=pooled2[:], in_=pooled2[:], func=AF.Relu,
                             bias=sh2[:], scale=sc2[:])

        # ================= conv3 =================
        im3_cm, im3_pool = _open_pool(tc, name="im3pool", bufs=1)
        im3 = im3_pool.tile([128, 32, 15, 14], BF16)
        for dw in range(2):
            for g2 in range(2):
                for rg in range(2):
                    b0 = 16 * rg + 8 * g2
                    nc.sync.dma_start(
                        im3[40 * dw:40 * dw + 40, b0:b0 + 8, :, :],
                        pooled2[64 * g2:64 * g2 + 40, rg, :, :, dw:dw + 14])

        for b2 in range(8):
            w_idx = b2
            bank = psum_conv.tile([128, 392], F32, tag="pb0", name=f"c3b_{w_idx}")
            for ct in range(2):
                b = 16 * ct + 2 * b2
                for s in range(2):
                    nc.tensor.matmul(
                        bank[64 * ct:64 * ct + 60, :],
                        lhsT=w3t[0:80, s, :],
                        rhs=im3[0:80, b:b + 2, s:s + 14, :],
                        start=(s == 0), stop=(s == 1),
                        tile_position=(0, 64 * ct),
                    )
            ych = chunks.tile([128, 2, 14, 14], BF16, tag="ych",
                              name=f"y3ch_{w_idx}")
            nc.scalar.activation(
                out=ych[:],
                in_=bank[:].rearrange("p (a b c) -> p a b c", a=2, b=14),
                func=AF.Identity, bias=b3v[:], scale=1.0,
                accum_out=s1a_3[:, w_idx:w_idx + 1])
            sq = chunks.tile([128, 2, 14, 14], BF16, tag="ysq",
                             name=f"y3sq_{w_idx}")
            nc.vector.tensor_mul(sq[:], ych[:], ych[:])
            nc.vector.reduce_sum(s2a_3[:, w_idx:w_idx + 1], sq[:], axis=AX.XYZ)
            p1 = chunks.tile([128, 2, 14, 7], BF16, tag="yp1",
                             name=f"y3p1_{w_idx}")
            nc.vector.tensor_max(out=p1[:], in0=ych[:, :, :, 0:14:2],
                                 in1=ych[:, :, :, 1:14:2])
            nc.vector.tensor_max(
                out=pooled3[:, w_idx],
                in0=p1[:, :, 0:14:2, :], in1=p1[:, :, 1:14:2, :])

        nc.vector.reduce_sum(s1f[:], s1a_3[:], axis=AX.X)
        nc.vector.reduce_sum(s2f[:], s2a_3[:], axis=AX.X)
        gstat3 = _stats_allgather(nc, persist, "bn3", 60, s1f[:], s2f[:],
                                  fold_groups=[0, 64])
        sc3, sh3 = _bn_scale_shift(nc, persist, "bn3", gstat3, bnp3, N3, eps_t)
        nc.scalar.activation(out=pooled3[:], in_=pooled3[:], func=AF.Relu,
                             bias=sh3[:], scale=sc3[:])

        # consolidate conv3 output for FC / LC
        for g2 in range(2):
            nc.sync.dma_start(
                h3c[0:60, 16 * g2:16 * g2 + 16, :],
                pooled3[64 * g2:64 * g2 + 60].rearrange(
                    "p a b c d -> p (a b) (c d)"))
        nc.sync.dma_start(h3c[64:124], h3c[0:60])
        for dw in range(2):
            nc.sync.dma_start(
                h3r[60 * dw:60 * dw + 60],
                h3c[0:60].rearrange("p b (i j) -> p b i j", i=7)[:, :, :, dw:dw + 6])

        # ================= LC layer =================
        for w4 in range(9):
            bank = psum_conv.tile([128, 4, BL], F32, tag="pb1", name=f"lcb_{w4}")
            for p4 in range(4):
                pos = w4 * 4 + p4
                i, j = divmod(pos, 6)
                for s in range(2):
                    nc.tensor.matmul(
                        bank[0:80, p4, :],
                        lhsT=lcw[0:120, pos, s, :],
                        rhs=h3r[0:120, :, i + s, j],
                        start=(s == 0), stop=(s == 1),
                        tile_position=(0, 0),
                    )
            nc.vector.scalar_tensor_tensor(
                out=lc_raw[0:80, w4 * 4:w4 * 4 + 4, :],
                in0=bank[0:80], scalar=1.0,
                in1=lcb[0:80, w4 * 4:w4 * 4 + 4, None].to_broadcast((80, 4, BL)),
                op0=ALU.mult, op1=ALU.add)

        s1_4 = persist.tile([128, 1], F32)
        s2_4 = persist.tile([128, 1], F32)
        nc.vector.reduce_sum(s1_4[0:80], lc_raw[0:80], axis=AX.XY)
        nc.vector.tensor_mul(lc_sq[0:80], lc_raw[0:80], lc_raw[0:80])
        nc.vector.reduce_sum(s2_4[0:80], lc_sq[0:80], axis=AX.XY)
        gstat4 = _stats_allgather(nc, persist, "bn4", 80,
                                  s1_4[:], s2_4[:], fold_groups=[0])
        sc4, sh4 = _bn_scale_shift(nc, persist, "bn4", gstat4, bnp4, N4, eps_t)
        nc.scalar.activation(out=lc_bn[0:80], in_=lc_raw[0:80], func=AF.Relu,
                             bias=sh4[0:80], scale=sc4[0:80])
        nc.sync.dma_start(lc_bn[64:104, :, :], lc_bn[40:80, :, :])

        # ================= FC =================
        im3_cm.__exit__(None, None, None)
        psum_conv_cm.__exit__(None, None, None)
        psum_fc_cm, psum_fc = _open_pool(tc, name="psfc", bufs=1, space="PSUM")
        acc0 = psum_fc.tile([128, 192], F32, name="fc_acc0")
        acc1 = psum_fc.tile([128, 192], F32, name="fc_acc1")
        accs = [acc0, acc1]
        for par in range(2):
            rt = 64 * par
            nt = 25 if par == 0 else 24
            for t in range(nt):
                ij = 2 * t + par
                for ct in range(4):
                    nc.tensor.matmul(
                        accs[par][32 * ct:32 * ct + 32, :],
                        lhsT=h3c[rt:rt + 60, :, ij],
                        rhs=fcw1[rt:rt + 60, t, 192 * ct:192 * ct + 192],
                        start=(t == 0), stop=False,
                        tile_position=(rt, 32 * ct),
                    )
        for kc in range(2):
            rt = 64 * kc
            for ij in range(36):
                for ct in range(4):
                    nc.tensor.matmul(
                        accs[kc][32 * ct:32 * ct + 32, :],
                        lhsT=lc_bn[rt:rt + 40, ij, :],
                        rhs=fcw2[rt:rt + 40, ij, 192 * ct:192 * ct + 192],
                        start=False, stop=(ij == 35),
                        tile_position=(rt, 32 * ct),
                    )
        # fold the two row-group accumulators; y5 holds pre-bias fc output
        t1 = persist.tile([128, 192], F32)
        nc.scalar.activation(out=t1[:], in_=acc1[:], func=AF.Copy)
        y5 = persist.tile([128, 192], F32)
        nc.vector.tensor_add(out=y5[:], in0=acc0[:], in1=t1[:])

        # bn5 stats via block-ones matmul (on pre-bias y; bias folded later)
        s5 = psum_fc.tile([4, 192], F32, name="s5")
        nc.tensor.matmul(s5[:], lhsT=ones4[:], rhs=y5[:], start=True, stop=True)
        y5q = persist.tile([128, 192], F32)
        nc.scalar.activation(out=y5q[:], in_=y5[:], func=AF.Square)
        s5q = psum_fc.tile([4, 192], F32, name="s5q")
        nc.tensor.matmul(s5q[:], lhsT=ones4[:], rhs=y5q[:], start=True, stop=True)
        st5 = persist.tile([4, 2, 192], F32)
        nc.scalar.activation(out=st5[:, 0, :], in_=s5[:], func=AF.Copy)
        nc.scalar.activation(out=st5[:, 1, :], in_=s5q[:], func=AF.Copy)

        cc5_in = nc.dram_tensor("cc_bn5_in", [4, 2, 192], F32)
        cc5_out = nc.dram_tensor("cc_bn5_out", [N_CORES, 4, 2, 192], F32,
                                 addr_space="Shared")
        nc.sync.dma_start(cc5_in[:], st5[:])
        nc.gpsimd.collective_compute(
            "AllGather", ALU.bypass,
            replica_groups=[list(range(N_CORES))],
            ins=[cc5_in[:]], outs=[cc5_out[:]],
        )
        g5all = persist.tile([4, 2, 192, N_CORES], F32)
        nc.sync.dma_start(g5all[:], cc5_out.rearrange("r g s f -> g s f r"))
        g5 = persist.tile([4, 2, 192], F32)
        nc.vector.reduce_sum(g5[:], g5all[:], axis=AX.X)

        # fold fc bias c into stats: S1' = S1 + 256c ; S2' = S2 + c*(2*S1 + 256c)
        s1p = persist.tile([4, 192], F32)
        nc.vector.scalar_tensor_tensor(
            out=s1p[:], in0=fcb4[:], scalar=256.0, in1=g5[:, 0, :],
            op0=ALU.mult, op1=ALU.add)
        t5a = persist.tile([4, 192], F32)
        nc.vector.tensor_add(out=t5a[:], in0=g5[:, 0, :], in1=s1p[:])
        t5b = persist.tile([4, 192], F32)
        nc.vector.tensor_mul(t5b[:], fcb4[:], t5a[:])
        s2p = persist.tile([4, 192], F32)
        nc.vector.tensor_add(out=s2p[:], in0=g5[:, 1, :], in1=t5b[:])

        mean5 = persist.tile([4, 192], F32)
        var5 = persist.tile([4, 192], F32)
        tmp5 = persist.tile([4, 192], F32)
        nc.vector.tensor_scalar_mul(mean5[:], s1p[:], 1.0 / N5)
        nc.vector.tensor_scalar_mul(var5[:], s2p[:], 1.0 / N5)
        nc.vector.tensor_mul(tmp5[:], mean5[:], mean5[:])
        nc.vector.tensor_sub(var5[:], var5[:], tmp5[:])
        nc.scalar.activation(out=tmp5[:], in_=var5[:], func=AF.Sqrt,
                             bias=eps_t[0:4], scale=1.0)
        nc.vector.reciprocal(out=tmp5[:], in_=tmp5[:])
        scale5 = persist.tile([4, 192], F32)
        shift5 = persist.tile([4, 192], F32)
        nc.vector.tensor_mul(scale5[:], bn5p[:, 0, :], tmp5[:])
        nc.vector.tensor_sub(tmp5[:], fcb4[:], mean5[:])
        nc.vector.tensor_mul(tmp5[:], tmp5[:], scale5[:])
        nc.vector.tensor_add(out=shift5[:], in0=bn5p[:, 1, :], in1=tmp5[:])

        # broadcast scale5/shift5 [4,192] -> [128,192] via a DRAM bounce
        sc5_d = nc.dram_tensor("sc5_scratch", [4, 192], F32)
        sh5_d = nc.dram_tensor("sh5_scratch", [4, 192], F32)
        nc.sync.dma_start(sc5_d[:], scale5[:])
        nc.sync.dma_start(sh5_d[:], shift5[:])
        scale5b = persist.tile([128, 192], F32)
        shift5b = persist.tile([128, 192], F32)
        for g in range(4):
            src_sc = bass.AP(tensor=sc5_d, offset=g * 192, ap=[[0, 32], [1, 192]])
            src_sh = bass.AP(tensor=sh5_d, offset=g * 192, ap=[[0, 32], [1, 192]])
            nc.gpsimd.dma_start(out=scale5b[32 * g:32 * g + 32, :], in_=src_sc)
            nc.gpsimd.dma_start(out=shift5b[32 * g:32 * g + 32, :], in_=src_sh)

        # apply bn5 + relu
        yb = persist.tile([128, 192], F32)
        nc.vector.tensor_mul(yb[:], y5[:], scale5b[:])
        nc.vector.tensor_add(out=yb[:], in0=yb[:], in1=shift5b[:])
        nc.vector.tensor_scalar_max(yb[:], yb[:], 0.0)

        # fuse: out[b, 12g+hh] = sum_s yb[32g+b, 16hh+s]*fw + fb
        fm = persist.tile([128, 192], F32)
        outs = persist.tile([128, 12], F32)
        nc.vector.tensor_mul(fm[:], yb[:], fw4[:])
        nc.vector.reduce_sum(
            outs[:], fm.rearrange("p (h s) -> p h s", s=16), axis=AX.X)
        nc.vector.tensor_add(out=outs[:], in0=outs[:], in1=fb4[:])
        for g in range(4):
            nc.sync.dma_start(out_d[:, 12 * g:12 * g + 12],
                              outs[32 * g:32 * g + 32, :])

        psum_fc_cm.__exit__(None, None, None)
        fcw2_cm.__exit__(None, None, None)
        fcw1_cm.__exit__(None, None, None)
        lcw_cm.__exit__(None, None, None)
        chunks_cm.__exit__(None, None, None)
        persist_cm.__exit__(None, None, None)

    return nc


# ---------------------------------------------------------------- host side

_NC_CACHE = None


def _get_nc():
    global _NC_CACHE
    if _NC_CACHE is None:
        _NC_CACHE = _split_excess_waits(build_nc())
    return _NC_CACHE


def _host_prep_shared(conv1_w, conv1_b, bn1_g, bn1_b,
                      conv2_w, conv2_b, bn2_g, bn2_b,
                      conv3_w, conv3_b, bn3_g, bn3_b,
                      lc_w, lc_b, bn4_g, bn4_b,
                      fc_w, fc_b, bn5_g, bn5_b,
                      fuse_w, fuse_b):
    d = {}
    bf = lambda a: np.ascontiguousarray(a.astype(np.float32)).astype(NPBF16)
    f3 = lambda a: np.ascontiguousarray(a).astype(np.float32)

    # wkT[dw*C+c, s, m] = conv_w[m, c, s, dw]
    d["w1t"] = bf(conv1_w.transpose(3, 1, 2, 0).reshape(9, 3, 20))
    d["w2t"] = bf(conv2_w.transpose(3, 1, 2, 0).reshape(40, 2, 40))
    d["w3t"] = bf(conv3_w.transpose(3, 1, 2, 0).reshape(80, 2, 60))

    lcw = lc_w[0]  # [80, 60, 6, 6, 4], patch idx = 2*dh + dw
    lcwt = np.zeros((120, 36, 2, 80), np.float32)
    for dw in range(2):
        for s in range(2):
            lcwt[60 * dw:60 * dw + 60, :, s, :] = (
                lcw[:, :, :, :, 2 * s + dw].reshape(80, 60, 36)
                .transpose(1, 2, 0))
    d["lcwt"] = lcwt.astype(NPBF16)
    d["lcb"] = f3(lc_b[0].reshape(80, 36))

    fw1 = fc_w[:, :2940].reshape(768, 60, 49)
    d["fcw1a"] = bf(fw1[:, :, 0::2].transpose(1, 2, 0))
    d["fcw1b"] = bf(fw1[:, :, 1::2].transpose(1, 2, 0))
    fw2 = fc_w[:, 2940:].reshape(768, 80, 36)
    d["fcw2a"] = bf(fw2[:, 0:40].transpose(1, 2, 0))
    d["fcw2b"] = bf(fw2[:, 40:80].transpose(1, 2, 0))

    def vec128(v, n, stride):
        o = np.zeros((128, 1), np.float32)
        for j in range(128 // stride):
            o[stride * j:stride * j + n, 0] = v
        return o

    d["b1v"] = vec128(conv1_b, 20, 32)
    d["b2v"] = vec128(conv2_b, 40, 64)
    d["b3v"] = vec128(conv3_b, 60, 64)

    def bnp128(g, b, n, stride):
        o = np.zeros((128, 2), np.float32)
        for j in range(128 // stride):
            o[stride * j:stride * j + n, 0] = g
            o[stride * j:stride * j + n, 1] = b
        return o

    d["bnp1"] = bnp128(bn1_g, bn1_b, 20, 32)
    d["bnp2"] = bnp128(bn2_g, bn2_b, 40, 64)
    d["bnp3"] = bnp128(bn3_g, bn3_b, 60, 64)
    o4 = np.zeros((128, 2), np.float32)
    o4[0:80, 0] = bn4_g
    o4[0:80, 1] = bn4_b
    d["bnp4"] = o4

    d["fcb4"] = f3(fc_b.reshape(4, 192))
    d["bn5p"] = f3(np.stack([bn5_g.reshape(4, 192), bn5_b.reshape(4, 192)],
                            axis=1))
    ff = fuse_w.reshape(4, 12, 16).reshape(4, 192)
    d["fw4"] = f3(np.repeat(ff, 32, axis=0))
    d["fb4"] = f3(np.repeat(fuse_b.reshape(4, 12), 32, axis=0))
    ones = np.zeros((128, 4), np.float32)
    for g in range(4):
        ones[32 * g:32 * g + 32, g] = 1.0
    d["ones4"] = ones.astype(np.float32)
    return d


def _host_prep_im1(x_shard):
    """x_shard [32, 3, 64, 64] f32 -> im1 [4, 9, 8, 64, 62] bf16,
    im1[g, 3*dw+c, b, i, j] = x[8g+b, c, i, j+dw]."""
    xs = x_shard.reshape(4, 8, 3, 64, 64)
    im1 = np.empty((4, 9, 8, 64, 62), np.float32)
    for dw in range(3):
        im1[:, 3 * dw:3 * dw + 3] = xs[:, :, :, :, dw:dw + 62].transpose(0, 2, 1, 3, 4)
    return im1.astype(NPBF16)


def kernel(**inputs):
    from concourse.bass_utils import run_bass_kernel_spmd
    x = np.asarray(inputs["x"], np.float32)
    shared = _host_prep_shared(
        **{k: np.asarray(v, np.float32) for k, v in inputs.items() if k != "x"})
    in_maps = []
    for r in range(N_CORES):
        m = dict(shared)
        m["im1"] = _host_prep_im1(x[BL * r:BL * (r + 1)])
        in_maps.append(m)
    nc = _get_nc()
    res = run_bass_kernel_spmd(nc, in_maps, core_ids=list(range(N_CORES)))
    out = np.concatenate([res.results[r]["out"] for r in range(N_CORES)], axis=0)
    return np.ascontiguousarray(out.astype(np.float32))


if __name__ == "__main__":
    sys.path.insert(0, '/root/problem')
    import reference
    inp = {k: np.asarray(v) for k, v in reference.setup_inputs().items()}
    got = kernel(**inp)
    exp = np.asarray(reference.reference(**inp))
    err = np.abs(got - exp).max() / (np.abs(exp).max() + 1e-9)
    print("out sample got:", got[0, :5])
    print("out sample exp:", exp[0, :5])
    print("rel err:", err)



# revision 19
# speedup vs baseline: 1.4592x; 1.4592x over previous
"""Trainium2 Bass kernel for nn_DDH_49246095016535 (dense CNN + LC + FC + fuse).

Data parallelism over 8 NeuronCores (32 samples each).  v2 design:

- conv1 runs as 64 block-diagonal matmuls: lhsT [108, 80] holds 4 diagonal
  (27, 20) blocks, so one instruction computes 4 samples' conv for the full
  27-deep (kh, kw, cin) contraction (kh replicated into partitions host-side).
- conv2/conv3 read their inputs directly from on-chip tensors laid out by
  cheap large-run SBUF->SBUF DMAs (kw replication via a flat shifted copy),
  killing the tiny-packet gather storms of the old kernel.
- Pooling runs on the raw f32 PSUM output (max commutes with the positive-
  scale BN affine + ReLU applied after pooling; conv biases cancel in
  training-mode BN exactly, so they are dropped).
- BN statistics are computed per PSUM tile either by DVE bn_stats or by a
  pair of Activation passes (Identity/Square with accum), balancing the two
  engines; cross-core reduction is a single small AllReduce per BN layer.
- conv3 writes its output with channels in partitions and all 32 samples in
  the free dim, which is directly the lhsT layout the FC and LC matmuls
  need (pos-pair packing via a +1/+7 shifted band copy).
"""
import sys

sys.path.insert(0, '/opt/trn_rl_repo')

import numpy as np
import ml_dtypes

import concourse.bass as bass
import concourse.tile as tile
import concourse.mybir as mybir

F32 = mybir.dt.float32
BF16 = mybir.dt.bfloat16
NPBF16 = ml_dtypes.bfloat16

N_CORES = 8
BL = 32          # samples per core
EPS = 1e-5

# batch-stat element counts over the full 256-sample batch
N1 = 256 * 62 * 62
N2 = 256 * 30 * 30
N3 = 256 * 14 * 14
N4 = 256 * 36
N5 = 256

AF = mybir.ActivationFunctionType
ALU = mybir.AluOpType
AX = mybir.AxisListType

MAX_DRAIN_WAITS = 1

# stats-route assignment: True -> Activation dual-pass, False -> DVE bn_stats
def _act1(idx):
    return (idx % 16) < 11


def _act2(idx):
    return (idx % 4) < 3


def _act3(idx):
    return (idx % 8) < 5


def _patched_drain_and_barrier(self, tick_clock, wait_clock):
    from concourse.vector_clock import ScopedClock
    nc = self.nc
    drain_inst = nc.sync.drain()
    wait_clock.add_sem_waits(drain_inst.ins, ScopedClock({None: tick_clock.global_clock}))
    si = drain_inst.ins.sync_info
    if si is not None and len(si.on_wait) > MAX_DRAIN_WAITS:
        waits = list(si.on_wait)
        drain_inst.ins.sync_info = mybir.SyncInfo(
            on_wait=waits[:MAX_DRAIN_WAITS], on_update=list(si.on_update))
        for k in range(MAX_DRAIN_WAITS, len(waits), MAX_DRAIN_WAITS):
            extra = nc.sync.drain()
            extra.ins.sync_info = mybir.SyncInfo(
                on_wait=waits[k:k + MAX_DRAIN_WAITS], on_update=[])
    nc.all_engine_barrier()
    assert self.sems is not None
    popped = nc._tile_sem_poison_stack.pop()
    assert popped is self._sem_poison
    nc.clear_and_free_semaphores(list(self.sems.allocated().values()))
    nc.all_engine_barrier()


tile.TileContext._drain_and_barrier = _patched_drain_and_barrier


def _split_excess_waits(nc, limit=1):
    """walrus codegen accepts at most one sync-wait per instruction; move the
    excess onto same-engine NoOps inserted immediately before."""
    nid = 0
    for f in nc.m.functions:
        for b in f.blocks:
            insts = b.instructions
            new_list = []
            changed = False
            for inst in insts:
                si = getattr(inst, "sync_info", None)
                if si is not None and len(si.on_wait) > limit and inst.engine is not None:
                    waits = list(si.on_wait)
                    keep, excess = waits[:limit], waits[limit:]
                    inst.sync_info = mybir.SyncInfo(
                        on_wait=keep, on_update=list(si.on_update))
                    for k in range(0, len(excess), limit):
                        nop = mybir.InstNoOp(name=f"I-wsplit-{nid}", ins=[], outs=[])
                        nid += 1
                        nop.engine = inst.engine
                        nop.sync_info = mybir.SyncInfo(
                            on_wait=excess[k:k + limit], on_update=[])
                        new_list.append(nop)
                    changed = True
                new_list.append(inst)
            if changed:
                insts[:] = new_list
    return nc


def _open_pool(tc, **kw):
    cm = tc.tile_pool(**kw)
    return cm, cm.__enter__()


def _merge_stats(nc, pool, name, p, s1a, s2a, na_cols, bst, nd_cols, nd_count):
    """Combine Act-route raw sums (s1a/s2a [p, na_cols]) with bn_stats-route
    aggregates (bst [p, nd_cols, 6]) into raw (S1, S2) sums st [p, 2]."""
    st = pool.tile([128, 2], F32, name=f"st_{name}")
    if na_cols:
        sa1 = pool.tile([128, 1], F32, name=f"sa1_{name}")
        sa2 = pool.tile([128, 1], F32, name=f"sa2_{name}")
        nc.vector.reduce_sum(sa1[0:p], s1a[0:p, 0:na_cols], axis=AX.X)
        nc.vector.reduce_sum(sa2[0:p], s2a[0:p, 0:na_cols], axis=AX.X)
    if nd_cols:
        mv = pool.tile([128, 2], F32, name=f"mv_{name}")
        nc.vector.bn_aggr(mv[0:p], bst[0:p, 0:nd_cols, :])
        # S1_D = nd * mean ; S2_D = nd * (var + mean^2)
        msq = pool.tile([128, 1], F32, name=f"msq_{name}")
        nc.vector.tensor_mul(msq[0:p], mv[0:p, 0:1], mv[0:p, 0:1])
        ex2 = pool.tile([128, 1], F32, name=f"ex2_{name}")
        nc.vector.tensor_add(out=ex2[0:p], in0=mv[0:p, 1:2], in1=msq[0:p])
        if na_cols:
            nc.vector.scalar_tensor_tensor(
                out=st[0:p, 0:1], in0=mv[0:p, 0:1], scalar=float(nd_count),
                in1=sa1[0:p], op0=ALU.mult, op1=ALU.add)
            nc.vector.scalar_tensor_tensor(
                out=st[0:p, 1:2], in0=ex2[0:p], scalar=float(nd_count),
                in1=sa2[0:p], op0=ALU.mult, op1=ALU.add)
        else:
            nc.vector.tensor_scalar_mul(st[0:p, 0:1], mv[0:p, 0:1], float(nd_count))
            nc.vector.tensor_scalar_mul(st[0:p, 1:2], ex2[0:p], float(nd_count))
    else:
        nc.vector.tensor_copy(out=st[0:p, 0:1], in_=sa1[0:p])
        nc.vector.tensor_copy(out=st[0:p, 1:2], in_=sa2[0:p])
    return st


def _bn_finalize(nc, pool, name, p, sums, bnp, n, eps_t):
    """sums [p, 2] = global raw (S1, S2); returns (scale [p,1], shift [p,1])."""
    mean = pool.tile([128, 1], F32, name=f"mean_{name}")
    var = pool.tile([128, 1], F32, name=f"var_{name}")
    tmp = pool.tile([128, 1], F32, name=f"tmp_{name}")
    scale = pool.tile([128, 1], F32, name=f"scale_{name}")
    shift = pool.tile([128, 1], F32, name=f"shift_{name}")
    inv_n = 1.0 / n
    nc.vector.tensor_scalar_mul(mean[0:p], sums[0:p, 0:1], inv_n)
    nc.vector.tensor_scalar_mul(var[0:p], sums[0:p, 1:2], inv_n)
    nc.vector.tensor_mul(tmp[0:p], mean[0:p], mean[0:p])
    nc.vector.tensor_sub(var[0:p], var[0:p], tmp[0:p])
    nc.scalar.activation(out=tmp[0:p], in_=var[0:p], func=AF.Sqrt,
                         bias=eps_t[0:p], scale=1.0)
    nc.vector.reciprocal(out=tmp[0:p], in_=tmp[0:p])
    nc.vector.tensor_mul(scale[0:p], bnp[0:p, 0:1], tmp[0:p])
    nc.vector.tensor_mul(tmp[0:p], mean[0:p], scale[0:p])
    nc.vector.tensor_sub(shift[0:p], bnp[0:p, 1:2], tmp[0:p])
    return scale, shift


def build_nc():
    nc = bass.Bass("TRN2", num_devices=N_CORES)

    im27_d = nc.dram_tensor("im27", [108, 8, 62, 62], BF16, kind="ExternalInput")
    w1_d = nc.dram_tensor("w1bd", [108, 80], BF16, kind="ExternalInput")
    w2_d = nc.dram_tensor("w2bd", [2, 80, 80], BF16, kind="ExternalInput")
    w3_d = nc.dram_tensor("w3bd", [2, 80, 60], BF16, kind="ExternalInput")
    lcw_d = nc.dram_tensor("lcw2", [120, 36, 2, 80], BF16, kind="ExternalInput")
    lcb_d = nc.dram_tensor("lcb80", [80, 36], F32, kind="ExternalInput")
    fw1_d = nc.dram_tensor("fcw1t", [120, 25, 768], BF16, kind="ExternalInput")
    fw2_d = nc.dram_tensor("fcw2t", [80, 36, 768], BF16, kind="ExternalInput")
    bnp1_d = nc.dram_tensor("bnp1", [80, 2], F32, kind="ExternalInput")
    bnp2_d = nc.dram_tensor("bnp2", [80, 2], F32, kind="ExternalInput")
    bnp3_d = nc.dram_tensor("bnp3", [60, 2], F32, kind="ExternalInput")
    bnp4_d = nc.dram_tensor("bnp4", [80, 2], F32, kind="ExternalInput")
    bn5p_d = nc.dram_tensor("bn5p", [4, 2, 192], F32, kind="ExternalInput")
    fw4_d = nc.dram_tensor("fw4", [128, 192], F32, kind="ExternalInput")
    fb4_d = nc.dram_tensor("fb4", [128, 12], F32, kind="ExternalInput")
    ones4_d = nc.dram_tensor("ones4", [128, 4], F32, kind="ExternalInput")
    out_d = nc.dram_tensor("out", [BL, 48], F32, kind="ExternalOutput")

    cc_in = {}
    cc_out = {}
    for name, rows in (("bn1", 80), ("bn2", 80), ("bn3", 60), ("bn4", 80)):
        cc_in[name] = nc.dram_tensor(f"cc_{name}_in", [rows, 2], F32)
        cc_out[name] = nc.dram_tensor(f"cc_{name}_out", [rows, 2], F32,
                                      addr_space="Shared")
    cc_in["bn5"] = nc.dram_tensor("cc_bn5_in", [4, 2, 192], F32)
    cc_out["bn5"] = nc.dram_tensor("cc_bn5_out", [4, 2, 192], F32,
                                   addr_space="Shared")
    sc5_d = nc.dram_tensor("sc5_scratch", [4, 192], F32)
    sh5_d = nc.dram_tensor("sh5_scratch", [4, 192], F32)

    RG = [list(range(N_CORES))]

    with tile.TileContext(nc) as tc:
        persist_cm, persist = _open_pool(tc, name="persist", bufs=1)
        chunks_cm, chunks = _open_pool(tc, name="chunks", bufs=4)
        psconv_cm, psconv = _open_pool(tc, name="psconv", bufs=6, space="PSUM")

        # ---------------- persistent params ----------------
        eps_t = persist.tile([128, 1], F32)
        nc.vector.memset(eps_t[:], EPS)
        bnp1 = persist.tile([80, 2], F32)
        nc.sync.dma_start(bnp1[:], bnp1_d[:])
        bnp2 = persist.tile([80, 2], F32)
        nc.sync.dma_start(bnp2[:], bnp2_d[:])
        bnp3 = persist.tile([60, 2], F32)
        nc.sync.dma_start(bnp3[:], bnp3_d[:])
        bnp4 = persist.tile([80, 2], F32)
        nc.sync.dma_start(bnp4[:], bnp4_d[:])
        lcb80 = persist.tile([80, 36], F32)
        nc.sync.dma_start(lcb80[:], lcb_d[:])
        bn5p = persist.tile([4, 2, 192], F32)
        nc.sync.dma_start(bn5p[:], bn5p_d[:])
        fw4 = persist.tile([128, 192], F32)
        nc.sync.dma_start(fw4[:], fw4_d[:])
        fb4 = persist.tile([128, 12], F32)
        nc.sync.dma_start(fb4[:], fb4_d[:])
        ones4 = persist.tile([128, 4], F32)
        nc.sync.dma_start(ones4[:], ones4_d[:])
        w1bd = persist.tile([108, 80], BF16)
        nc.sync.dma_start(w1bd[:], w1_d[:])
        w2bd = persist.tile([80, 2, 80], BF16)
        # dram [2, 80, 80] -> partition dim first in SBUF
        nc.sync.dma_start(w2bd[:], w2_d.rearrange("s p m -> p s m"))
        w3bd = persist.tile([80, 2, 60], BF16)
        nc.sync.dma_start(w3bd[:], w3_d.rearrange("s p m -> p s m"))
        lcw2 = persist.tile([120, 36, 2, 80], BF16)
        nc.sync.dma_start(lcw2[:], lcw_d[:])

        # persistent activations / stats
        pooled3f = persist.tile([60, 32, 7, 7], F32)
        pooled3 = persist.tile([60, 32, 7, 7], BF16)
        hfc = persist.tile([120, 32, 49], BF16)
        hlc = persist.tile([120, 32, 49], BF16)
        lc_raw = persist.tile([80, 36, 32], BF16)
        lc_bn = persist.tile([80, 36, 32], BF16)
        s1a1 = persist.tile([80, 48], F32)
        s2a1 = persist.tile([80, 48], F32)
        bst1 = persist.tile([80, 24, 6], F32)
        s1a2 = persist.tile([80, 24], F32)
        s2a2 = persist.tile([80, 24], F32)
        bst2 = persist.tile([80, 8, 6], F32)
        s1a3 = persist.tile([60, 12], F32)
        s2a3 = persist.tile([60, 12], F32)
        bst3 = persist.tile([60, 8, 6], F32)
        bst4 = persist.tile([80, 3, 6], F32)

        # ================= conv1 =================
        pool1_cm, pool1_pool = _open_pool(tc, name="pool1pool", bufs=1, side="right")
        pooled1 = pool1_pool.tile([80, 8, 31, 31], BF16)
        pool1f_cm, pool1f_pool = _open_pool(tc, name="pool1fpool", bufs=1, side="right")
        pooled1f = pool1f_pool.tile([80, 8, 31, 31], F32)

        im27_cm, im27_pool = _open_pool(tc, name="im27pool", bufs=1, side="right")
        im27 = im27_pool.tile([108, 8, 62, 62], BF16)
        for b in range(8):
            nc.sync.dma_start(im27[:, b], im27_d[:, b])

        na1 = nd1 = 0
        NA1 = ND1 = 0
        for b in range(8):
            for blk in range(8):
                idx = b * 8 + blk
                nr = 8 if blk < 7 else 6
                nf = nr * 62
                pc = psconv.tile([80, 8, 62], F32, tag="pc", name=f"c1_{idx}")
                nc.tensor.matmul(
                    pc[:, :nr, :], lhsT=w1bd[:], rhs=im27[:, b, 8 * blk:8 * blk + nr, :],
                    start=True, stop=True)
                flat = pc[:].rearrange("p a b -> p (a b)")
                if _act1(idx):
                    d1 = chunks.tile([80, 8, 62], BF16, tag="d1", name=f"c1d1_{idx}")
                    nc.scalar.activation(
                        out=d1[:].rearrange("p a b -> p (a b)")[:, :nf],
                        in_=flat[:, :nf], func=AF.Identity,
                        accum_out=s1a1[:, na1:na1 + 1])
                    d2 = chunks.tile([80, 8, 62], BF16, tag="d2", name=f"c1d2_{idx}")
                    nc.scalar.activation(
                        out=d2[:].rearrange("p a b -> p (a b)")[:, :nf],
                        in_=flat[:, :nf], func=AF.Square,
                        accum_out=s2a1[:, na1:na1 + 1])
                    na1 += 1
                    NA1 += nf
                else:
                    nc.vector.bn_stats(out=bst1[:, nd1, :], in_=flat[:, :nf])
                    nd1 += 1
                    ND1 += nf
                wt = chunks.tile([80, 8, 31], F32, tag="wt", name=f"c1w_{idx}")
                nc.vector.reduce_max(
                    wt[:, :nr, :],
                    pc[:, :nr, :].rearrange("p a (x two) -> p a x two", two=2),
                    axis=AX.X)
                nc.vector.tensor_max(
                    out=pooled1f[:, b, 4 * blk:4 * blk + nr // 2, :],
                    in0=wt[:, 0:nr:2, :], in1=wt[:, 1:nr:2, :])

        st1 = _merge_stats(nc, persist, "bn1", 80, s1a1, s2a1, na1, bst1, nd1, ND1)
        nc.sync.dma_start(cc_in["bn1"][:], st1[0:80])
        nc.gpsimd.collective_compute(
            "AllReduce", ALU.add, replica_groups=RG,
            ins=[cc_in["bn1"][:]], outs=[cc_out["bn1"][:]])
        # fold the 4 sample-groups (rows 20g+oc all get sum over g)
        gall1 = persist.tile([80, 4, 2], F32)
        for g in range(4):
            src = bass.AP(tensor=cc_out["bn1"], offset=40 * g,
                          ap=[[0, 4], [2, 20], [1, 2]])
            nc.gpsimd.dma_start(out=gall1[:, g, :], in_=src)
        g1 = persist.tile([80, 2], F32)
        nc.vector.reduce_sum(g1[:], gall1[:].rearrange("p g v -> p v g"), axis=AX.X)
        sc1, sh1 = _bn_finalize(nc, persist, "bn1", 80, g1, bnp1, N1, eps_t)

        # free im27; stage FC weights into the freed space
        im27_cm.__exit__(None, None, None)
        fcw1_cm, fcw1_pool = _open_pool(tc, name="fcw1pool", bufs=1)
        fcw1 = fcw1_pool.tile([120, 25, 768], BF16)
        nc.sync.dma_start(fcw1[:], fw1_d[:])
        # bn1 apply (per-b chunks) + remap into conv2 layout
        p1d_cm, p1d_pool = _open_pool(tc, name="p1dpool", bufs=1)
        p1d = p1d_pool.tile([80, 16, 31, 31], BF16)
        for b in range(8):
            nc.scalar.activation(out=pooled1[:, b], in_=pooled1f[:, b],
                                 func=AF.Relu, bias=sh1[0:80], scale=sc1[0:80])
        pool1f_cm.__exit__(None, None, None)
        for g in range(4):
            G, gp = g // 2, g % 2
            for b in range(8):
                b16 = 8 * gp + b
                nc.sync.dma_start(
                    p1d[40 * G:40 * G + 20, b16], pooled1[20 * g:20 * g + 20, b])
                nc.sync.dma_start(
                    p1d[40 * G + 20:40 * G + 40, b16].rearrange(
                        "p y x -> p (y x)")[:, 0:960],
                    pooled1[20 * g:20 * g + 20, b].rearrange(
                        "p y x -> p (y x)")[:, 1:961])

        # ================= conv2 =================
        pool1_cm.__exit__(None, None, None)
        p2r_cm, p2r_pool = _open_pool(tc, name="p2rpool", bufs=1, side="right")
        p2raw = p2r_pool.tile([80, 16, 15, 15], BF16)
        p2rf_cm, p2rf_pool = _open_pool(tc, name="p2rfpool", bufs=1, side="right")
        p2rawf = p2rf_pool.tile([80, 16, 15, 15], F32)

        na2 = nd2 = 0
        NA2 = ND2 = 0
        for b16 in range(16):
            for h2 in range(2):
                idx = b16 * 2 + h2
                nr = 16 if h2 == 0 else 14
                nf = nr * 30
                pc = psconv.tile([80, 16, 30], F32, tag="pc", name=f"c2_{idx}")
                for s in range(2):
                    nc.tensor.matmul(
                        pc[:, :nr, :], lhsT=w2bd[:, s, :],
                        rhs=p1d[:, b16, 16 * h2 + s:16 * h2 + s + nr, 0:30],
                        start=(s == 0), stop=(s == 1))
                flat = pc[:].rearrange("p a b -> p (a b)")
                if _act2(idx):
                    d1 = chunks.tile([80, 16, 30], BF16, tag="d1", name=f"c2d1_{idx}")
                    nc.scalar.activation(
                        out=d1[:].rearrange("p a b -> p (a b)")[:, :nf],
                        in_=flat[:, :nf], func=AF.Identity,
                        accum_out=s1a2[:, na2:na2 + 1])
                    d2 = chunks.tile([80, 16, 30], BF16, tag="d2", name=f"c2d2_{idx}")
                    nc.scalar.activation(
                        out=d2[:].rearrange("p a b -> p (a b)")[:, :nf],
                        in_=flat[:, :nf], func=AF.Square,
                        accum_out=s2a2[:, na2:na2 + 1])
                    na2 += 1
                    NA2 += nf
                else:
                    nc.vector.bn_stats(out=bst2[:, nd2, :], in_=flat[:, :nf])
                    nd2 += 1
                    ND2 += nf
                wt = chunks.tile([80, 16, 15], F32, tag="wt", name=f"c2w_{idx}")
                nc.vector.reduce_max(
                    wt[:, :nr, :],
                    pc[:, :nr, :].rearrange("p a (x two) -> p a x two", two=2),
                    axis=AX.X)
                nc.vector.tensor_max(
                    out=p2rawf[:, b16, 8 * h2:8 * h2 + nr // 2, :],
                    in0=wt[:, 0:nr:2, :], in1=wt[:, 1:nr:2, :])

        st2 = _merge_stats(nc, persist, "bn2", 80, s1a2, s2a2, na2, bst2, nd2, ND2)
        nc.sync.dma_start(cc_in["bn2"][:], st2[0:80])
        nc.gpsimd.collective_compute(
            "AllReduce", ALU.add, replica_groups=RG,
            ins=[cc_in["bn2"][:]], outs=[cc_out["bn2"][:]])
        gall2 = persist.tile([80, 2, 2], F32)
        for G in range(2):
            src = bass.AP(tensor=cc_out["bn2"], offset=80 * G,
                          ap=[[0, 2], [2, 40], [1, 2]])
            nc.gpsimd.dma_start(out=gall2[:, G, :], in_=src)
        g2 = persist.tile([80, 2], F32)
        nc.vector.reduce_sum(g2[:], gall2[:].rearrange("p g v -> p v g"), axis=AX.X)
        sc2, sh2 = _bn_finalize(nc, persist, "bn2", 80, g2, bnp2, N2, eps_t)

        # bn2 apply + remap into conv3 layout [80=40dw+c, 32s, 15, 15]
        p1d_cm.__exit__(None, None, None)
        fcw2_cm, fcw2_pool = _open_pool(tc, name="fcw2pool", bufs=1)
        fcw2 = fcw2_pool.tile([80, 36, 768], BF16)
        nc.sync.dma_start(fcw2[:], fw2_d[:])
        p2d_cm, p2d_pool = _open_pool(tc, name="p2dpool", bufs=1)
        p2d = p2d_pool.tile([80, 32, 15, 15], BF16)
        for q in range(4):
            nc.scalar.activation(out=p2raw[:, 4 * q:4 * q + 4],
                                 in_=p2rawf[:, 4 * q:4 * q + 4],
                                 func=AF.Relu, bias=sh2[0:80], scale=sc2[0:80])
        p2rf_cm.__exit__(None, None, None)
        for G in range(2):
            for q in range(4):
                src = p2raw[40 * G:40 * G + 40, 4 * q:4 * q + 4]
                nc.sync.dma_start(p2d[0:40, 16 * G + 4 * q:16 * G + 4 * q + 4], src)
                nc.sync.dma_start(
                    p2d[40:80, 16 * G + 4 * q:16 * G + 4 * q + 4].rearrange(
                        "p b y x -> p (b y x)")[:, 0:899],
                    src.rearrange("p b y x -> p (b y x)")[:, 1:900])

        p2r_cm.__exit__(None, None, None)

        # ================= conv3 =================
        na3 = nd3 = 0
        NA3 = ND3 = 0
        for t in range(16):
            idx = t
            pc = psconv.tile([60, 2, 14, 14], F32, tag="pc", name=f"c3_{idx}")
            for s in range(2):
                nc.tensor.matmul(
                    pc[:], lhsT=w3bd[:, s, :],
                    rhs=p2d[:, 2 * t:2 * t + 2, s:s + 14, 0:14],
                    start=(s == 0), stop=(s == 1))
            flat = pc[:].rearrange("p a b c -> p (a b c)")
            if _act3(idx):
                d1 = chunks.tile([60, 2, 14, 14], BF16, tag="d1", name=f"c3d1_{idx}")
                nc.scalar.activation(
                    out=d1[:].rearrange("p a b c -> p (a b c)"),
                    in_=flat, func=AF.Identity, accum_out=s1a3[:, na3:na3 + 1])
                d2 = chunks.tile([60, 2, 14, 14], BF16, tag="d2", name=f"c3d2_{idx}")
                nc.scalar.activation(
                    out=d2[:].rearrange("p a b c -> p (a b c)"),
                    in_=flat, func=AF.Square, accum_out=s2a3[:, na3:na3 + 1])
                na3 += 1
                NA3 += 392
            else:
                nc.vector.bn_stats(out=bst3[:, nd3, :], in_=flat)
                nd3 += 1
                ND3 += 392
            wt = chunks.tile([60, 2, 14, 7], F32, tag="wt", name=f"c3w_{idx}")
            nc.vector.reduce_max(
                wt[:],
                pc[:].rearrange("p a b (x two) -> p a b x two", two=2),
                axis=AX.X)
            nc.vector.tensor_max(
                out=pooled3f[:, 2 * t:2 * t + 2], in0=wt[:, :, 0:14:2, :],
                in1=wt[:, :, 1:14:2, :])

        st3 = _merge_stats(nc, persist, "bn3", 60, s1a3, s2a3, na3, bst3, nd3, ND3)
        nc.sync.dma_start(cc_in["bn3"][:], st3[0:60])
        nc.gpsimd.collective_compute(
            "AllReduce", ALU.add, replica_groups=RG,
            ins=[cc_in["bn3"][:]], outs=[cc_out["bn3"][:]])
        g3 = persist.tile([60, 2], F32)
        nc.sync.dma_start(g3[:], cc_out["bn3"][:])
        sc3, sh3 = _bn_finalize(nc, persist, "bn3", 60, g3, bnp3, N3, eps_t)
        nc.scalar.activation(out=pooled3[:], in_=pooled3f[:], func=AF.Relu,
                             bias=sh3[0:60], scale=sc3[0:60])

        # conv3 -> FC/LC boundary: base + shifted band copies
        p3flat = pooled3[:].rearrange("p s y x -> p (s y x)")
        nc.sync.dma_start(hfc[0:60], pooled3[:])
        nc.sync.dma_start(
            hfc[60:120].rearrange("p s q -> p (s q)")[:, 0:1567], p3flat[:, 1:1568])
        nc.sync.dma_start(hlc[0:60], pooled3[:])
        nc.sync.dma_start(
            hlc[60:120].rearrange("p s q -> p (s q)")[:, 0:1561], p3flat[:, 7:1568])

        # ================= FC x1 (starts as soon as hfc ready) ============
        p2d_cm.__exit__(None, None, None)
        psconv_cm.__exit__(None, None, None)
        psfc_cm, psfc = _open_pool(tc, name="psfc", bufs=1, space="PSUM")
        acc = psfc.tile([128, 192], F32, name="fc_acc")
        for t in range(25):
            for fg in range(4):
                if t < 24:
                    nc.tensor.matmul(
                        acc[32 * fg:32 * fg + 32, :],
                        lhsT=hfc[0:120, :, 2 * t],
                        rhs=fcw1[0:120, t, 192 * fg:192 * fg + 192],
                        start=(t == 0), stop=False,
                        tile_position=(0, 32 * fg))
                else:
                    nc.tensor.matmul(
                        acc[32 * fg:32 * fg + 32, :],
                        lhsT=hfc[0:60, :, 48],
                        rhs=fcw1[0:60, t, 192 * fg:192 * fg + 192],
                        start=False, stop=False,
                        tile_position=(0, 32 * fg))

        # ================= LC =================
        lcps_cm, lcps = _open_pool(tc, name="lcps", bufs=1, space="PSUM")
        for ch in range(3):
            lcp = lcps.tile([80, 12, 32], F32, name=f"lcp_{ch}")
            for p12 in range(12):
                pos = 12 * ch + p12
                i, j = divmod(pos, 6)
                for dw in range(2):
                    nc.tensor.matmul(
                        lcp[:, p12, :], lhsT=lcw2[:, pos, dw, :],
                        rhs=hlc[:, :, 7 * i + j + dw],
                        start=(dw == 0), stop=(dw == 1))
            # add the locally-connected bias before stats (it does not cancel)
            nc.vector.scalar_tensor_tensor(
                out=lc_raw[:, 12 * ch:12 * ch + 12, :], in0=lcp[:], scalar=1.0,
                in1=lcb80[:, 12 * ch:12 * ch + 12, None].to_broadcast((80, 12, 32)),
                op0=ALU.mult, op1=ALU.add)
            nc.vector.bn_stats(
                out=bst4[:, ch, :],
                in_=lc_raw[:, 12 * ch:12 * ch + 12, :].rearrange("p a b -> p (a b)"))

        mv4 = persist.tile([80, 2], F32)
        nc.vector.bn_aggr(mv4[:], bst4[:])
        st4 = persist.tile([80, 2], F32)
        msq4 = persist.tile([80, 1], F32)
        nc.vector.tensor_mul(msq4[:], mv4[:, 0:1], mv4[:, 0:1])
        nc.vector.tensor_scalar_mul(st4[:, 0:1], mv4[:, 0:1], float(36 * BL))
        nc.vector.tensor_add(out=st4[:, 1:2], in0=mv4[:, 1:2], in1=msq4[:])
        nc.vector.tensor_scalar_mul(st4[:, 1:2], st4[:, 1:2], float(36 * BL))
        nc.sync.dma_start(cc_in["bn4"][:], st4[0:80])
        nc.gpsimd.collective_compute(
            "AllReduce", ALU.add, replica_groups=RG,
            ins=[cc_in["bn4"][:]], outs=[cc_out["bn4"][:]])
        g4 = persist.tile([80, 2], F32)
        nc.sync.dma_start(g4[:], cc_out["bn4"][:])
        sc4, sh4 = _bn_finalize(nc, persist, "bn4", 80, g4, bnp4, N4, eps_t)
        nc.scalar.activation(out=lc_bn[:], in_=lc_raw[:], func=AF.Relu,
                             bias=sh4[0:80], scale=sc4[0:80])
        lcps_cm.__exit__(None, None, None)

        # ================= FC x2 =================
        for pos in range(36):
            for fg in range(4):
                nc.tensor.matmul(
                    acc[32 * fg:32 * fg + 32, :],
                    lhsT=lc_bn[0:80, pos, :],
                    rhs=fcw2[0:80, pos, 192 * fg:192 * fg + 192],
                    start=False, stop=(pos == 35),
                    tile_position=(0, 32 * fg))

        # ================= bn5 + fuse =================
        y5 = persist.tile([128, 192], F32)
        nc.scalar.activation(out=y5[:], in_=acc[:], func=AF.Copy)
        y5q = persist.tile([128, 192], F32)
        nc.scalar.activation(out=y5q[:], in_=y5[:], func=AF.Square)
        s5 = psfc.tile([4, 192], F32, name="s5")
        nc.tensor.matmul(s5[:], lhsT=ones4[:], rhs=y5[:], start=True, stop=True)
        s5q = psfc.tile([4, 192], F32, name="s5q")
        nc.tensor.matmul(s5q[:], lhsT=ones4[:], rhs=y5q[:], start=True, stop=True)
        st5 = persist.tile([4, 2, 192], F32)
        nc.scalar.activation(out=st5[:, 0, :], in_=s5[:], func=AF.Copy)
        nc.scalar.activation(out=st5[:, 1, :], in_=s5q[:], func=AF.Copy)
        nc.sync.dma_start(cc_in["bn5"][:], st5[:])
        nc.gpsimd.collective_compute(
            "AllReduce", ALU.add, replica_groups=RG,
            ins=[cc_in["bn5"][:]], outs=[cc_out["bn5"][:]])
        g5 = persist.tile([4, 2, 192], F32)
        nc.sync.dma_start(g5[:], cc_out["bn5"][:])

        mean5 = persist.tile([4, 192], F32)
        var5 = persist.tile([4, 192], F32)
        tmp5 = persist.tile([4, 192], F32)
        nc.vector.tensor_scalar_mul(mean5[:], g5[:, 0, :], 1.0 / N5)
        nc.vector.tensor_scalar_mul(var5[:], g5[:, 1, :], 1.0 / N5)
        nc.vector.tensor_mul(tmp5[:], mean5[:], mean5[:])
        nc.vector.tensor_sub(var5[:], var5[:], tmp5[:])
        nc.scalar.activation(out=tmp5[:], in_=var5[:], func=AF.Sqrt,
                             bias=eps_t[0:4], scale=1.0)
        nc.vector.reciprocal(out=tmp5[:], in_=tmp5[:])
        scale5 = persist.tile([4, 192], F32)
        shift5 = persist.tile([4, 192], F32)
        nc.vector.tensor_mul(scale5[:], bn5p[:, 0, :], tmp5[:])
        nc.vector.tensor_mul(tmp5[:], mean5[:], scale5[:])
        nc.vector.tensor_sub(shift5[:], bn5p[:, 1, :], tmp5[:])

        # broadcast scale5/shift5 [4,192] -> [128,192] via a DRAM bounce
        nc.sync.dma_start(sc5_d[:], scale5[:])
        nc.sync.dma_start(sh5_d[:], shift5[:])
        scale5b = persist.tile([128, 192], F32)
        shift5b = persist.tile([128, 192], F32)
        for g in range(4):
            src_sc = bass.AP(tensor=sc5_d, offset=g * 192, ap=[[0, 32], [1, 192]])
            src_sh = bass.AP(tensor=sh5_d, offset=g * 192, ap=[[0, 32], [1, 192]])
            nc.gpsimd.dma_start(out=scale5b[32 * g:32 * g + 32, :], in_=src_sc)
            nc.gpsimd.dma_start(out=shift5b[32 * g:32 * g + 32, :], in_=src_sh)

        yb = persist.tile([128, 192], F32)
        nc.vector.tensor_mul(yb[:], y5[:], scale5b[:])
        nc.vector.tensor_add(out=yb[:], in0=yb[:], in1=shift5b[:])
        nc.vector.tensor_scalar_max(yb[:], yb[:], 0.0)

        fm = persist.tile([128, 192], F32)
        outs = persist.tile([128, 12], F32)
        nc.vector.tensor_mul(fm[:], yb[:], fw4[:])
        nc.vector.reduce_sum(
            outs[:], fm[:].rearrange("p (h s) -> p h s", s=16), axis=AX.X)
        nc.vector.tensor_add(out=outs[:], in0=outs[:], in1=fb4[:])
        for g in range(4):
            nc.sync.dma_start(out_d[:, 12 * g:12 * g + 12],
                              outs[32 * g:32 * g + 32, :])

        psfc_cm.__exit__(None, None, None)
        fcw2_cm.__exit__(None, None, None)
        fcw1_cm.__exit__(None, None, None)
        chunks_cm.__exit__(None, None, None)
        persist_cm.__exit__(None, None, None)

    return nc


# ---------------------------------------------------------------- host side

_NC_CACHE = None


def _get_nc():
    global _NC_CACHE
    if _NC_CACHE is None:
        _NC_CACHE = _split_excess_waits(build_nc())
    return _NC_CACHE


def _host_prep_shared(conv1_w, conv1_b, bn1_g, bn1_b,
                      conv2_w, conv2_b, bn2_g, bn2_b,
                      conv3_w, conv3_b, bn3_g, bn3_b,
                      lc_w, lc_b, bn4_g, bn4_b,
                      fc_w, fc_b, bn5_g, bn5_b,
                      fuse_w, fuse_b):
    d = {}
    f3 = lambda a: np.ascontiguousarray(a).astype(np.float32)

    # conv1 block-diagonal lhsT [108, 80]: block g at rows 27g, cols 20g;
    # row within block = 9*kh + 3*kw + cin, col = oc
    blk1 = conv1_w.transpose(2, 3, 1, 0).reshape(27, 20).astype(np.float32)
    w1bd = np.zeros((108, 80), np.float32)
    for g in range(4):
        w1bd[27 * g:27 * g + 27, 20 * g:20 * g + 20] = blk1
    d["w1bd"] = w1bd.astype(NPBF16)

    # conv2 lhsT per kh step s: [80, 80], 2 diagonal (40, 40) blocks;
    # rows 40G + 20*kw + cin, cols 40G + oc
    w2bd = np.zeros((2, 80, 80), np.float32)
    for s in range(2):
        blk = conv2_w[:, :, s, :].transpose(2, 1, 0).reshape(40, 40)
        for G in range(2):
            w2bd[s, 40 * G:40 * G + 40, 40 * G:40 * G + 40] = blk
    d["w2bd"] = w2bd.astype(NPBF16)

    # conv3 lhsT per kh step s: [80, 60], rows 40*kw + cin, cols oc
    w3bd = np.zeros((2, 80, 60), np.float32)
    for s in range(2):
        w3bd[s] = conv3_w[:, :, s, :].transpose(2, 1, 0).reshape(80, 60)
    d["w3bd"] = w3bd.astype(NPBF16)

    # LC weights [120, 36, 2, 80]: rows 60*kh + cin, slice (pos, kw), cols oc
    lcw = lc_w[0].reshape(80, 60, 36, 4)  # oc, c, pos, patch(2kh+kw)
    lcw2 = np.zeros((120, 36, 2, 80), np.float32)
    for kh in range(2):
        for kw in range(2):
            lcw2[60 * kh:60 * kh + 60, :, kw, :] = (
                lcw[:, :, :, 2 * kh + kw].transpose(1, 2, 0))
    d["lcw2"] = lcw2.astype(NPBF16)
    d["lcb80"] = f3(lc_b[0].reshape(80, 36))

    # FC x1 weights [120, 25, 768]: rows c -> pos 2t, rows 60+c -> pos 2t+1
    fw1 = fc_w[:, :2940].reshape(768, 60, 49)
    fcw1t = np.zeros((120, 25, 768), np.float32)
    fcw1t[0:60, :, :] = fw1[:, :, 0::2].transpose(1, 2, 0)
    fcw1t[60:120, 0:24, :] = fw1[:, :, 1::2].transpose(1, 2, 0)
    d["fcw1t"] = fcw1t.astype(NPBF16)
    # FC x2 weights [80, 36, 768]
    d["fcw2t"] = np.ascontiguousarray(
        fc_w[:, 2940:].reshape(768, 80, 36).transpose(1, 2, 0)).astype(NPBF16)

    d["bnp1"] = f3(np.stack([np.tile(bn1_g, 4), np.tile(bn1_b, 4)], axis=1))
    d["bnp2"] = f3(np.stack([np.tile(bn2_g, 2), np.tile(bn2_b, 2)], axis=1))
    d["bnp3"] = f3(np.stack([bn3_g, bn3_b], axis=1))
    d["bnp4"] = f3(np.stack([bn4_g, bn4_b], axis=1))
    d["bn5p"] = f3(np.stack([bn5_g.reshape(4, 192), bn5_b.reshape(4, 192)],
                            axis=1))
    ff = fuse_w.reshape(4, 12, 16).reshape(4, 192)
    d["fw4"] = f3(np.repeat(ff, 32, axis=0))
    d["fb4"] = f3(np.repeat(fuse_b.reshape(4, 12), 32, axis=0))
    ones = np.zeros((128, 4), np.float32)
    for g in range(4):
        ones[32 * g:32 * g + 32, g] = 1.0
    d["ones4"] = ones
    return d


def _host_prep_im1(x_shard):
    """x_shard [32, 3, 64, 64] f32 -> im27 [108, 8, 62, 62] bf16,
    im27[27g + 9kh + 3kw + c, b, r, j] = x[8g+b, c, r+kh, j+kw]."""
    xs = x_shard.reshape(4, 8, 3, 64, 64)
    im27 = np.empty((4, 27, 8, 62, 62), np.float32)
    for kh in range(3):
        for kw in range(3):
            im27[:, 9 * kh + 3 * kw:9 * kh + 3 * kw + 3] = (
                xs[:, :, :, kh:kh + 62, kw:kw + 62].transpose(0, 2, 1, 3, 4))
    return im27.reshape(108, 8, 62, 62).astype(NPBF16)


def kernel(**inputs):
    from concourse.bass_utils import run_bass_kernel_spmd
    x = np.asarray(inputs["x"], np.float32)
    shared = _host_prep_shared(
        **{k: np.asarray(v, np.float32) for k, v in inputs.items() if k != "x"})
    in_maps = []
    for r in range(N_CORES):
        m = dict(shared)
        m["im27"] = _host_prep_im1(x[BL * r:BL * (r + 1)])
        in_maps.append(m)
    nc = _get_nc()
    res = run_bass_kernel_spmd(nc, in_maps, core_ids=list(range(N_CORES)))
    out = np.concatenate([res.results[r]["out"] for r in range(N_CORES)], axis=0)
    return np.ascontiguousarray(out.astype(np.float32))


if __name__ == "__main__":
    sys.path.insert(0, '/root/problem')
    import reference
    inp = {k: np.asarray(v) for k, v in reference.setup_inputs().items()}
    got = kernel(**inp)
    exp = np.asarray(reference.reference(**inp))
    err = np.abs(got - exp).max() / (np.abs(exp).max() + 1e-9)
    print("out sample got:", got[0, :5])
    print("out sample exp:", exp[0, :5])
    print("rel err:", err)
